# revision 13
# baseline (speedup 1.0000x reference)
import numpy as np

# nn_Attention_84756884619813 — sparse attention with token pruning.
# Strategy: data-parallel over batch B=16 across 8 NeuronCores (2 rows/core)
# via jax.pmap (XLA->neuronx-cc); weights replicated. The heavy compute (QKV,
# per-head attention, masked-exp normalization, AV, output projection, the
# head-mean logits and the 5-step power iteration) runs on-device. Only the
# O(B*K log K) argsort/top-k/scatter tail runs on host.

B, NP, C, H = 16, 576, 1024, 16
N = NP + 1           # 577 queries (CLS + patches)
T = N + 1            # 578 keys (incl. memory token)
HD = C // H
SCALE = HD ** -0.5
K_KEPT = 512
RETAIN = int(K_KEPT * 0.8)   # 409
TAU = 0.5
EPS = 1e-6
NCORES = 8
RB = B // NCORES

_PMAPPED = None
DEVICE_USED = False


def _build_pmap():
    import jax
    import jax.numpy as jnp

    def fwd(x, mem, mask_all, key_mask, Wqkv, Wproj):
        # x: (RB, N, C), mem: (RB, 1, C), mask_all: (RB, N), key_mask: (RB, T)
        tokens = jnp.concatenate([x, mem], axis=1)                  # (RB, T, C)
        qkv = (tokens @ Wqkv).reshape(RB, T, 3, H, HD).transpose(2, 0, 3, 1, 4)
        q, k, v = qkv[0], qkv[1], qkv[2]                            # (RB, H, T, HD)
        q = q[:, :, :-1]                                            # (RB, H, N, HD)
        attn = jnp.einsum('bhqd,bhkd->bhqk', q, k) * SCALE          # (RB, H, N, T)

        # importance over the square CLS+patch block
        m = jnp.mean(attn[:, :, :, :-1] / TAU, axis=1)              # (RB, N, N)
        a = jnp.exp(m) * mask_all[:, None, :]
        Mrow = a / a.sum(axis=-1, keepdims=True)
        dist = jnp.full((RB, 1, N), 1.0 / N, Mrow.dtype)
        for _ in range(5):
            dist = dist @ Mrow
        importance = dist[:, 0, 1:]                                 # (RB, N-1)

        e = jnp.exp(attn) * key_mask[:, None, None, :]
        attn_n = (e + EPS / T) / (e.sum(axis=-1, keepdims=True) + EPS)
        out = jnp.einsum('bhqk,bhkd->bhqd', attn_n, v)
        out = out.transpose(0, 2, 1, 3).reshape(RB, N, C) @ Wproj   # (RB, N, C)
        return out, importance

    return jax.pmap(fwd, in_axes=(0, 0, 0, 0, None, None),
                    devices=jax.devices()[:NCORES])


def _forward_numpy(x, memory_token, mask_all, key_mask, Wqkv, Wproj):
    out = np.empty((B, N, C), np.float32)
    importance = np.empty((B, N - 1), np.float32)
    for b in range(B):
        tokens = np.concatenate([x[b], memory_token[b]], axis=0)    # (T, C)
        qkv = (tokens @ Wqkv).reshape(T, 3, H, HD).transpose(1, 2, 0, 3)
        q, k, v = qkv[0], qkv[1], qkv[2]                            # (H, T, HD)
        q = q[:, :-1]                                               # (H, N, HD)
        attn = np.einsum('hqd,hkd->hqk', q, k) * SCALE              # (H, N, T)

        m = np.mean(attn[:, :, :-1] / TAU, axis=0)                  # (N, N)
        a = np.exp(m) * mask_all[b][None, :]
        Mrow = a / a.sum(axis=-1, keepdims=True)
        dist = np.full((1, N), 1.0 / N, Mrow.dtype)
        for _ in range(5):
            dist = dist @ Mrow
        importance[b] = dist[0, 1:]

        e = np.exp(attn) * key_mask[b][None, None, :]
        attn_n = (e + EPS / T) / (e.sum(axis=-1, keepdims=True) + EPS)
        o = np.einsum('hqk,hkd->hqd', attn_n, v)
        out[b] = o.transpose(1, 0, 2).reshape(N, C) @ Wproj
    return out, importance


def kernel(x, memory_token, token_mask, Wqkv, Wproj, bproj):
    global _PMAPPED
    x = np.asarray(x, np.float32)
    memory_token = np.asarray(memory_token, np.float32)
    token_mask = np.asarray(token_mask, np.float32)
    Wqkv = np.asarray(Wqkv, np.float32)
    Wproj = np.asarray(Wproj, np.float32)
    bproj = np.asarray(bproj, np.float32)

    ones1 = np.ones((B, 1), np.float32)
    mask_all = np.concatenate([ones1, token_mask], axis=1)          # (B, N)
    key_mask = np.concatenate([mask_all, ones1], axis=1)            # (B, T)

    try:
        if _PMAPPED is None:
            _PMAPPED = _build_pmap()
        xs = x.reshape(NCORES, RB, N, C)
        ms = memory_token.reshape(NCORES, RB, 1, C)
        mas = mask_all.reshape(NCORES, RB, N)
        kms = key_mask.reshape(NCORES, RB, T)
        out_d, imp_d = _PMAPPED(xs, ms, mas, kms, Wqkv, Wproj)
        out = np.asarray(out_d, np.float32).reshape(B, N, C)
        importance = np.asarray(imp_d, np.float32).reshape(B, N - 1)
        globals()['DEVICE_USED'] = True
    except Exception:
        out, importance = _forward_numpy(x, memory_token, mask_all, key_mask,
                                         Wqkv, Wproj)

    out = (out + bproj).astype(np.float32)

    # top-k pruning among kept tokens (stable sorts match jnp.argsort)
    selected = np.argsort(1.0 - token_mask, axis=1, kind="stable")[:, :K_KEPT]
    imp_kept = np.take_along_axis(importance, selected, axis=1)
    inds = np.argsort(-imp_kept, axis=1, kind="stable")[:, :RETAIN]
    imp_inds = np.take_along_axis(selected, inds, axis=1)
    next_mask = np.zeros((B, NP), np.float32)
    next_mask[np.arange(B)[:, None], imp_inds] = 1.0
    return out, next_mask, importance.astype(np.float32)
